# revision 15
# baseline (speedup 1.0000x reference)
"""Trainium2 Bass kernel for ClassificationRNN (embed -> LSTM(T=300) -> dense -> softmax).

Strategy: data-parallel over batch (8 cores x 64 rows). Everything transposed
("layout C"): state h,c kept as [H, B] so the recurrence matmul streams the
small batch operand while weights are the stationary operand, elementwise runs
on full 128 partitions, and the final dense layer consumes h tiles directly as
lhsT without any per-step transposes.

Per step t (all on-chip):
  gates^T [4H=8x128 chunks, 64] accumulate in one PSUM bank [128, 512]:
      bias outer-product MM + x-part MM (from pre-gathered x^T) + h-part MMs
  sigmoid(i,f,o) / tanh(g) on ScalarE straight from PSUM,
  c = f*c + i*g ; h = o*tanh(c) on VectorE,
  dens^T accumulation: dens[64,256] += h_t^T slices @ W3^T slices (W3 streamed
  from HBM during the scan).

Gate order is host-permuted to (i, f, o, g) so one sigmoid op covers cols 0:384.
"""

import os

import numpy as np

import concourse.bass as bass
import concourse.mybir as mybir
import concourse.tile as tile
from concourse import bacc
from concourse.bass_utils import run_bass_kernel_spmd
from concourse.masks import make_identity

B, T, V, D, H, DENSE, C = 512, 300, 50000, 128, 256, 256, 14
NCORES = 8
BL = B // NCORES  # 64 batch rows per core
G4 = 4 * H  # 1024 gates
F32 = mybir.dt.float32
BF16 = mybir.dt.bfloat16

# matmul operand dtype for the recurrence / dense weights (f32 or bf16)
MM_BF16 = os.environ.get("KERNEL_MM_DTYPE", "f32") == "bf16"
MMDT = BF16 if MM_BF16 else F32
NP_MMDT = np.dtype("bfloat16") if MM_BF16 else np.float32


def build_bass(t_steps: int = T):
    TS = t_steps
    NTOK = TS * BL  # tokens per core
    NGT = NTOK // 128  # gather tiles (2 timesteps each)
    nc = bacc.Bacc(None, target_bir_lowering=True, debug=False)

    ids_d = nc.dram_tensor("ids", [128, NGT], mybir.dt.int32, kind="ExternalInput")
    emb_d = nc.dram_tensor("emb", [V + 1, D], F32, kind="ExternalInput")
    wcat_d = nc.dram_tensor("wcat_t", [H + D, G4], MMDT, kind="ExternalInput")
    bres_d = nc.dram_tensor("bres", [8, 128], F32, kind="ExternalInput")
    sel_d = nc.dram_tensor("sel", [8, 8 * BL], F32, kind="ExternalInput")
    w3_d = nc.dram_tensor("w3_t", [TS * H, DENSE], MMDT, kind="ExternalInput")
    w4_d = nc.dram_tensor("w4_p", [128, 2 * C], F32, kind="ExternalInput")
    b3_d = nc.dram_tensor("b3_p", [128, 2], F32, kind="ExternalInput")
    b4_d = nc.dram_tensor("b4_p", [C, 1], F32, kind="ExternalInput")
    out_d = nc.dram_tensor("out", [BL, C], F32, kind="ExternalOutput")

    with tile.TileContext(nc) as tc:
        with tc.tile_pool(name="const", bufs=1) as cp:
            ident = cp.tile([128, 128], F32)
            make_identity(nc, ident[:])
            ids_s = cp.tile([128, NGT], mybir.dt.int32)
            nc.sync.dma_start(out=ids_s[:], in_=ids_d[:])
            wcat_s = cp.tile([128, 3 * G4], MMDT)
            for k in range(3):
                nc.sync.dma_start(
                    out=wcat_s[:, k * G4 : (k + 1) * G4],
                    in_=wcat_d[k * 128 : (k + 1) * 128, :],
                )
            bres_s = cp.tile([8, 128], F32)
            nc.sync.dma_start(out=bres_s[:], in_=bres_d[:])
            sel_s = cp.tile([8, 8 * BL], F32)
            nc.sync.dma_start(out=sel_s[:], in_=sel_d[:])
            w4_s = cp.tile([128, 2 * C], F32)
            nc.sync.dma_start(out=w4_s[:], in_=w4_d[:])
            b3_s = cp.tile([128, 2], F32)
            nc.sync.dma_start(out=b3_s[:], in_=b3_d[:])
            b4_s = cp.tile([C, 1], F32)
            nc.sync.dma_start(out=b4_s[:], in_=b4_d[:])

            xt_hist = cp.tile([128, NTOK], MMDT)  # x^T, col = t*BL + b
            c_st = cp.tile([128, 2 * BL], F32)  # c state packed [h-chunk, b]
            nc.vector.memset(c_st[:], 0.0)

            # ---- prologue: gather embeddings and transpose into xt_hist ----
            with (
                tc.tile_pool(name="gath", bufs=4) as gp,
                tc.tile_pool(name="tp", bufs=4, space="PSUM") as tpp,
            ):
                for j in range(NGT):
                    xg = gp.tile([128, 128], F32)
                    nc.gpsimd.indirect_dma_start(
                        out=xg[:],
                        out_offset=None,
                        in_=emb_d[:],
                        in_offset=bass.IndirectOffsetOnAxis(
                            ap=ids_s[:, j : j + 1], axis=0
                        ),
                    )
                    xp = tpp.tile([128, 128], F32)
                    nc.tensor.transpose(out=xp[:], in_=xg[:], identity=ident[:])
                    dst = xt_hist[:, j * 128 : (j + 1) * 128]
                    if j % 2 == 0:
                        nc.vector.tensor_copy(out=dst, in_=xp[:])
                    else:
                        nc.scalar.copy(out=dst, in_=xp[:])

            # ---- scan ----
            with (
                tc.tile_pool(name="gpsum", bufs=2, space="PSUM") as gpp,
                tc.tile_pool(name="dpsum", bufs=1, space="PSUM") as dpp,
                tc.tile_pool(name="sc", bufs=2) as sp,
                tc.tile_pool(name="w3p", bufs=6) as w3p,
                tc.tile_pool(name="hp", bufs=2) as hp,
            ):
                dens_ps = dpp.tile([BL, DENSE], F32)
                h_prev = None
                for t in range(TS):
                    gps = gpp.tile([128, 8 * BL], F32)
                    # bias outer product fills whole bank, starts accum group
                    nc.tensor.matmul(
                        out=gps[:], lhsT=bres_s[:], rhs=sel_s[:],
                        start=True, stop=False, skip_group_check=True,
                    )
                    # x-part: K = D
                    xs = xt_hist[:, t * BL : (t + 1) * BL]
                    for m in range(8):
                        nc.tensor.matmul(
                            out=gps[:, m * BL : (m + 1) * BL],
                            lhsT=wcat_s[:, 2 * G4 + m * 128 : 2 * G4 + (m + 1) * 128],
                            rhs=xs,
                            start=False, stop=(t == 0), skip_group_check=True,
                        )
                    # h-part: K = H (2 chunks)
                    if t > 0:
                        for kc in range(2):
                            hs = h_prev[:, kc * BL : (kc + 1) * BL]
                            for m in range(8):
                                nc.tensor.matmul(
                                    out=gps[:, m * BL : (m + 1) * BL],
                                    lhsT=wcat_s[:, kc * G4 + m * 128 : kc * G4 + (m + 1) * 128],
                                    rhs=hs,
                                    start=False, stop=(kc == 1),
                                    skip_group_check=True,
                                )
                    # activations straight from PSUM
                    gs = sp.tile([128, 6 * BL], F32, tag="gs")
                    nc.scalar.activation(
                        out=gs[:], in_=gps[:, : 6 * BL],
                        func=mybir.ActivationFunctionType.Sigmoid,
                    )
                    tg = sp.tile([128, 2 * BL], F32, tag="tg")
                    nc.scalar.activation(
                        out=tg[:], in_=gps[:, 6 * BL : 8 * BL],
                        func=mybir.ActivationFunctionType.Tanh,
                    )
                    # c = f*c + i*g ; h = o*tanh(c)
                    ig = sp.tile([128, 2 * BL], F32, tag="ig")
                    nc.vector.tensor_tensor(
                        out=ig[:], in0=gs[:, : 2 * BL], in1=tg[:],
                        op=mybir.AluOpType.mult,
                    )
                    nc.vector.tensor_tensor(
                        out=c_st[:], in0=gs[:, 2 * BL : 4 * BL], in1=c_st[:],
                        op=mybir.AluOpType.mult,
                    )
                    nc.vector.tensor_tensor(
                        out=c_st[:], in0=c_st[:], in1=ig[:],
                        op=mybir.AluOpType.add,
                    )
                    tc_t = sp.tile([128, 2 * BL], F32, tag="tc")
                    nc.scalar.activation(
                        out=tc_t[:], in_=c_st[:],
                        func=mybir.ActivationFunctionType.Tanh,
                    )
                    h_new = hp.tile([128, 2 * BL], MMDT, tag="h")
                    nc.vector.tensor_tensor(
                        out=h_new[:], in0=gs[:, 4 * BL : 6 * BL], in1=tc_t[:],
                        op=mybir.AluOpType.mult,
                    )
                    # fused dense: dens += h_t^T chunks @ W3^T rows
                    for kc in range(2):
                        wt = w3p.tile([128, DENSE], MMDT, tag="w3")
                        nc.sync.dma_start(
                            out=wt[:],
                            in_=w3_d[t * H + kc * 128 : t * H + (kc + 1) * 128, :],
                        )
                        nc.tensor.matmul(
                            out=dens_ps[:],
                            lhsT=h_new[:, kc * BL : (kc + 1) * BL],
                            rhs=wt[:],
                            start=(t == 0 and kc == 0),
                            stop=(t == TS - 1 and kc == 1),
                            skip_group_check=True,
                        )
                    h_prev = h_new

                # ---- epilogue: relu(dens+b3) -> transpose -> W4 -> softmax ----
                with (
                    tc.tile_pool(name="ep", bufs=1) as ep,
                    tc.tile_pool(name="epp", bufs=1, space="PSUM") as epp,
                ):
                    dens_s = ep.tile([BL, DENSE], F32)
                    nc.vector.tensor_copy(out=dens_s[:], in_=dens_ps[:])
                    densT = ep.tile([128, 2 * BL], F32)
                    for kc in range(2):
                        dp = epp.tile([128, BL], F32, tag="dtp")
                        nc.tensor.transpose(
                            out=dp[:],
                            in_=dens_s[:, kc * 128 : (kc + 1) * 128],
                            identity=ident[:BL, :BL],
                        )
                        # relu(dens + b3): dense dim now on partitions
                        nc.scalar.activation(
                            out=densT[:, kc * BL : (kc + 1) * BL], in_=dp[:],
                            func=mybir.ActivationFunctionType.Relu,
                            bias=b3_s[:, kc : kc + 1],
                        )
                    logT_ps = epp.tile([C, BL], F32, tag="lps")
                    for kc in range(2):
                        nc.tensor.matmul(
                            out=logT_ps[:],
                            lhsT=w4_s[:, kc * C : (kc + 1) * C],
                            rhs=densT[:, kc * BL : (kc + 1) * BL],
                            start=(kc == 0), stop=(kc == 1),
                            skip_group_check=True,
                        )
                    logT_s = ep.tile([C, BL], F32)
                    nc.scalar.activation(
                        out=logT_s[:], in_=logT_ps[:],
                        func=mybir.ActivationFunctionType.Identity,
                        bias=b4_s[:, 0:1],
                    )
                    log_ps = epp.tile([BL, C], F32, tag="lg")
                    nc.tensor.transpose(
                        out=log_ps[:], in_=logT_s[:], identity=ident[:C, :C]
                    )
                    negmax = ep.tile([BL, 1], F32)
                    nc.vector.tensor_reduce(
                        out=negmax[:], in_=log_ps[:],
                        axis=mybir.AxisListType.X, op=mybir.AluOpType.max,
                        negate=True,
                    )
                    exp_s = ep.tile([BL, C], F32)
                    sumexp = ep.tile([BL, 1], F32)
                    nc.scalar.activation(
                        out=exp_s[:], in_=log_ps[:],
                        func=mybir.ActivationFunctionType.Exp,
                        bias=negmax[:, 0:1],
                        accum_out=sumexp[:, 0:1],
                    )
                    rec = ep.tile([BL, 1], F32)
                    nc.vector.reciprocal(out=rec[:], in_=sumexp[:])
                    probs = ep.tile([BL, C], F32)
                    nc.vector.tensor_scalar_mul(
                        out=probs[:], in0=exp_s[:], scalar1=rec[:, 0:1]
                    )
                    nc.sync.dma_start(out=out_d[:], in_=probs[:])
    nc.compile()
    return nc


_NC_CACHE = {}


def _get_nc(t_steps):
    if t_steps not in _NC_CACHE:
        _NC_CACHE[t_steps] = build_bass(t_steps)
    return _NC_CACHE[t_steps]


def _host_prep(inputs, t_steps):
    ids = np.asarray(inputs["inputs_arrays"]).astype(np.int32)[:, :t_steps]
    emb = np.asarray(inputs["emb"], dtype=np.float32)
    W_ih = np.asarray(inputs["W_ih"], dtype=np.float32)
    W_hh = np.asarray(inputs["W_hh"], dtype=np.float32)
    b_ih = np.asarray(inputs["b_ih"], dtype=np.float32)
    b_hh = np.asarray(inputs["b_hh"], dtype=np.float32)
    W3 = np.asarray(inputs["W3"], dtype=np.float32)
    b3 = np.asarray(inputs["b3"], dtype=np.float32)
    W4 = np.asarray(inputs["W4"], dtype=np.float32)
    b4 = np.asarray(inputs["b4"], dtype=np.float32)

    # gate permutation i,f,g,o -> i,f,o,g
    perm = np.concatenate(
        [np.arange(0, H), np.arange(H, 2 * H), np.arange(3 * H, 4 * H),
         np.arange(2 * H, 3 * H)]
    )
    wcat = np.concatenate([W_hh[perm], W_ih[perm]], axis=1)  # [4H, H+D]
    wcat_t = np.ascontiguousarray(wcat.T).astype(NP_MMDT)  # [H+D, 4H]
    b_cat = (b_ih + b_hh)[perm].astype(np.float32)
    bres = np.ascontiguousarray(b_cat.reshape(8, 128))
    sel = np.repeat(np.eye(8, dtype=np.float32), BL, axis=1)  # [8, 8*BL]
    # W3 columns follow original gate-free ordering [b, t*H + h] (no perm!)
    W3s = W3.reshape(DENSE, T, H)[:, :t_steps, :].reshape(DENSE, t_steps * H)
    w3_t = np.ascontiguousarray(W3s.T).astype(NP_MMDT)  # [TS*H, DENSE]
    w4_t = np.ascontiguousarray(W4.T)  # [DENSE, C]
    w4_p = np.ascontiguousarray(
        w4_t.reshape(2, 128, C).transpose(1, 0, 2).reshape(128, 2 * C)
    )
    b3_p = np.ascontiguousarray(b3.reshape(2, 128).T)
    b4_p = np.ascontiguousarray(b4.reshape(C, 1))

    in_maps = []
    for core in range(NCORES):
        ids_c = ids[core * BL : (core + 1) * BL]  # [BL, TS]
        # tile layout: [128, NGT], element (p, j) = ids_c[p % BL, 2j + p // BL]
        ids_a = np.ascontiguousarray(
            ids_c.reshape(BL, t_steps // 2, 2).transpose(2, 0, 1).reshape(128, -1)
        )
        in_maps.append(
            {
                "ids": ids_a,
                "emb": emb,
                "wcat_t": wcat_t,
                "bres": bres,
                "sel": sel,
                "w3_t": w3_t,
                "w4_p": w4_p,
                "b3_p": b3_p,
                "b4_p": b4_p,
            }
        )
    return in_maps


def run(inputs, t_steps=T, **spmd_kwargs):
    nc = _get_nc(t_steps)
    in_maps = _host_prep(inputs, t_steps)
    res = run_bass_kernel_spmd(nc, in_maps, core_ids=list(range(NCORES)), **spmd_kwargs)
    out = np.concatenate([r["out"] for r in res.results], axis=0)
    return out, res


def kernel(**inputs) -> np.ndarray:
    out, _ = run(inputs, T)
    return out
